# revision 3
# baseline (speedup 1.0000x reference)
"""Trainium2 Bass kernel (final: v4a + all-scalar exp) for nn_AttentionCore64: softmax(Q@K^T)@V (raw exp,
no scaling), B=2 H=16 S=2048 D=64, f32 in/out. B*H sharded over 8 cores.

v3 vs v2: DMA instruction count cut ~7x (~40 vs ~280; HWDGE descriptor-gen is
625ns/DMA on a single shared resource):
- XBAR transposes batched: one dma_start_transpose covers N 128-col blocks
  (out[:, j, :] = in[:, j*128:(j+1)*128].T blockwise semantics, verified).
- loads are per-head 3D DMAs covering many o-tile groups; the fp32->bf16
  convert does the head-interleave relayout (engine APs can be 4D).
- fin batched per 2 chunks: one XBAR transpose [80,2048]->[128,16,80], one
  normalize, one output DMA per head per 2 chunks.

v4: PV emission delayed PVD=3 iterations (deque). With delay 1 the pipeline
period was forced to >= sem+exp latency (~1.3us/t -> 166us total): PV(t) sat
on PE right behind QK(t+1) but had to wait for exp(t). With PVD=3 the exp
latency amortizes 3x; the 3 PVs pending at chunk end flush into the next
chunk's first 3 iterations (which have no PV of their own yet).
"""

import numpy as np
from contextlib import ExitStack

import concourse.tile as tile
import concourse.mybir as mybir
from concourse import bacc
from concourse.bass_utils import run_bass_kernel_spmd

B, H, S, D = 2, 16, 2048, 64
NCORES = 8
HPC = (B * H) // NCORES  # 4 heads per core

P = 128
CH = 512            # queries per chunk
NCH = S // CH       # 4 chunks per pair
NT = S // P         # 16 key tiles
NO = S // P         # 16 query o-tiles
DT = mybir.dt
AF = mybir.ActivationFunctionType
OP = mybir.AluOpType

PAIRS = [(0, 1), (2, 3)]
VW = 80             # padded PV weight cols: 64 v + 1 ones + 15 zeros

A16 = float(128.0 / np.log(2.0))
B16 = 16256.0 - 7.30  # calibrated: E[schraudolph/exp] = 1.0003

# exp engine assignment per t (16 per chunk): 9 ScalarE / 7 DVE
SCALAR_T = frozenset(range(16))


def build(reps=None):
    nc = bacc.Bacc("TRN2", target_bir_lowering=False, debug=False)
    q_ext = nc.dram_tensor("q", [HPC, S, D], DT.float32, kind="ExternalInput").ap()
    k_ext = nc.dram_tensor("k", [HPC, S, D], DT.float32, kind="ExternalInput").ap()
    v_ext = nc.dram_tensor("v", [HPC, S, D], DT.float32, kind="ExternalInput").ap()
    out_ext = nc.dram_tensor("out", [HPC, S, D], DT.float32, kind="ExternalOutput").ap()

    with tile.TileContext(nc) as tc, ExitStack() as ctx:
        if reps is not None:
            ctx.enter_context(tc.For_i(0, reps))
        sb = ctx.enter_context(tc.tile_pool(name="sb", bufs=2))
        pr = ctx.enter_context(tc.tile_pool(name="pr", bufs=4))
        ps_sc = ctx.enter_context(tc.tile_pool(name="ps_sc", bufs=3, space="PSUM"))
        ps_out = ctx.enter_context(tc.tile_pool(name="ps_out", bufs=1, space="PSUM"))

        # ---------------- per-pair state ----------------
        def alloc_pair(pi):
            st = {"pi": pi}
            # nat tiles are HEAD-MAJOR so each load DMA is 3D (one head)
            for nm in ("q_nat", "k_nat", "v_nat"):
                st[nm] = sb.tile([P, 2, NO, D], DT.float32, tag=nm, name=nm)
            # bf tiles are O-MAJOR: [o, head, d] so a 128-col block = one
            # o-tile of the packed pair (relayout happens in the convert)
            for nm in ("qbf", "kbf"):
                st[nm] = sb.tile([P, NO, 2, D], DT.bfloat16, tag=nm, name=nm)
            for nm in ("qT", "kT"):
                st[nm] = sb.tile([P, NO, P], DT.bfloat16, tag=nm, name=nm)
            st["vx"] = sb.tile([P, NT, 2, VW], DT.bfloat16, tag="vx", name="vx")
            return st

        def load_head(st, which, hi, g0, gn):
            """One 3D DMA: o-tile groups [g0, g0+gn) of one head."""
            hA, _ = PAIRS[st["pi"]]
            ext = {"q": q_ext, "k": k_ext, "v": v_ext}[which]
            dst = st[{"q": "q_nat", "k": "k_nat", "v": "v_nat"}[which]]
            sl = slice(g0 * 4, (g0 + gn) * 4)
            nc.sync.dma_start(
                dst[:, hi, sl, :],
                ext[hA + hi].rearrange("(o p) d -> p o d", p=P)[:, sl, :],
            )

        def convert_groups(st, which, g0, gn, eng):
            """fp32 head-major -> bf16 o-major for groups [g0, g0+gn)."""
            src = st[{"q": "q_nat", "k": "k_nat"}[which]]
            dst = st[{"q": "qbf", "k": "kbf"}[which]]
            sl = slice(g0 * 4, (g0 + gn) * 4)
            s_ap = src[:, :, sl, :].rearrange("p h o d -> p o h d")
            if eng == "s":
                nc.scalar.copy(dst[:, sl, :, :], s_ap)
            else:
                nc.vector.tensor_copy(dst[:, sl, :, :], s_ap)

        def vx_groups(st, g0, gn):
            """Build vx = [v | 1 | zeros] bf16 for groups [g0, g0+gn) (GPSIMD)."""
            sl = slice(g0 * 4, (g0 + gn) * 4)
            for hi in (0, 1):
                nc.gpsimd.tensor_copy(
                    st["vx"][:, sl, hi, :D], st["v_nat"][:, hi, sl, :]
                )
            nc.gpsimd.memset(st["vx"][:, sl, :, D], 1.0)
            nc.gpsimd.memset(st["vx"][:, sl, :, D + 1 :], 0.0)

        def tr_groups(st, which, g0, gn):
            """One XBAR DMA transposing o-tiles [g0*4, (g0+gn)*4) blockwise."""
            src = st[{"q": "qbf", "k": "kbf"}[which]]
            dst = st[{"q": "qT", "k": "kT"}[which]]
            sl = slice(g0 * 4, (g0 + gn) * 4)
            nc.sync.dma_start_transpose(dst[:, sl, :], src[:, sl, :, :])

        # ---------------- finalization (per 2-chunk group) ----------------
        # outT col-blocks ordered [A-c0 | A-c1 | B-c0 | B-c1] so each head's
        # transposed o-tiles are contiguous for a single 3D output DMA.
        def emit_fin_copy(fin):
            out_ps = fin["out_ps"]
            cc = fin["c"] % 2
            if cc == 0:
                fin2 = {"pi": fin["pi"], "c0": fin["c"]}
                fin2["outT"] = sb.tile(
                    [VW, 2, 2, CH], DT.bfloat16, tag="outT", name="outT"
                )
            else:
                fin2 = fin["fin2"]
            outT = fin2["outT"]
            nc.scalar.copy(outT[:, 0, cc, :], out_ps[:, :CH])
            nc.vector.tensor_copy(outT[:, 1, cc, :], out_ps[:, CH:])
            return fin2

        def emit_fin_tr(fin2):
            finT = sb.tile([P, 2, 8, VW], DT.bfloat16, tag="finT", name="finT")
            nc.sync.dma_start_transpose(finT[:], fin2["outT"][:])
            fin2["finT"] = finT

        def emit_fin_out(fin2):
            finT = fin2["finT"]  # [128, hi, (cc j), 80]
            hA, _ = PAIRS[fin2["pi"]]
            c0 = fin2["c0"]
            recip = sb.tile([P, 2, 8, 1], DT.float32, tag="recip", name="recip")
            nc.vector.reciprocal(recip[:], finT[:, :, :, D : D + 1])
            outn = sb.tile([P, 2, 8, D], DT.float32, tag="outn", name="outn")
            nc.vector.tensor_tensor(
                outn[:],
                finT[:, :, :, :D],
                recip[:].to_broadcast((P, 2, 8, D)),
                OP.mult,
            )
            for hi in (0, 1):
                nc.sync.dma_start(
                    out_ext[hA + hi].rearrange("(o p) d -> p o d", p=P)[
                        :, c0 * 4 : c0 * 4 + 8, :
                    ],
                    outn[:, hi, :, :],
                )

        def emit_pv(st, out_ps, probs, t):
            nc.tensor.matmul(
                out_ps[:, :CH], st["vx"][:, t, 0, :], probs[:, :CH],
                start=(t == 0), stop=(t == NT - 1),
            )
            nc.tensor.matmul(
                out_ps[:, CH:], st["vx"][:, t, 1, :], probs[:, CH:],
                start=(t == 0), stop=(t == NT - 1),
            )

        # ---------------- pair-setup emission scheduling ----------------
        def p0_stream_ops(st):
            return [
                lambda: load_head(st, "k", 0, 2, 2),
                lambda: load_head(st, "k", 1, 2, 2),
                lambda: convert_groups(st, "k", 2, 1, "s"),
                lambda: convert_groups(st, "k", 3, 1, "v"),
                lambda: tr_groups(st, "k", 2, 2),
                lambda: load_head(st, "v", 0, 2, 2),
                lambda: load_head(st, "v", 1, 2, 2),
                lambda: vx_groups(st, 2, 1),
                lambda: vx_groups(st, 3, 1),
                lambda: load_head(st, "q", 0, 1, 3),
                lambda: load_head(st, "q", 1, 1, 3),
                lambda: convert_groups(st, "q", 1, 1, "v"),
                lambda: convert_groups(st, "q", 2, 1, "s"),
                lambda: convert_groups(st, "q", 3, 1, "v"),
                lambda: tr_groups(st, "q", 1, 3),
            ]

        def p1_stream_ops(st):
            return [
                lambda: load_head(st, "k", 0, 0, 4),
                lambda: load_head(st, "k", 1, 0, 4),
                lambda: convert_groups(st, "k", 0, 2, "s"),
                lambda: convert_groups(st, "k", 2, 2, "v"),
                lambda: tr_groups(st, "k", 0, 4),
                lambda: load_head(st, "v", 0, 0, 4),
                lambda: load_head(st, "v", 1, 0, 4),
                lambda: vx_groups(st, 0, 2),
                lambda: vx_groups(st, 2, 2),
                lambda: load_head(st, "q", 0, 0, 4),
                lambda: load_head(st, "q", 1, 0, 4),
                lambda: convert_groups(st, "q", 0, 2, "s"),
                lambda: convert_groups(st, "q", 2, 2, "v"),
                lambda: tr_groups(st, "q", 0, 4),
            ]

        # ---------------- prologue: pair 0 groups 0-1 of k/v, group 0 of q --
        st_cur = alloc_pair(0)
        load_head(st_cur, "k", 0, 0, 2)
        load_head(st_cur, "k", 1, 0, 2)
        load_head(st_cur, "q", 0, 0, 1)
        load_head(st_cur, "q", 1, 0, 1)
        convert_groups(st_cur, "k", 0, 1, "s")
        convert_groups(st_cur, "q", 0, 1, "v")
        tr_groups(st_cur, "q", 0, 1)
        convert_groups(st_cur, "k", 1, 1, "s")
        tr_groups(st_cur, "k", 0, 2)
        load_head(st_cur, "v", 0, 0, 2)
        load_head(st_cur, "v", 1, 0, 2)
        vx_groups(st_cur, 0, 2)
        pending_setup = p0_stream_ops(st_cur)

        from collections import deque

        PVD = 2
        st_next = None
        pending_pv = deque()
        fin = None
        fin2_done = None  # 2-chunk fin group ready for tr/out

        for pi in range(len(PAIRS)):
            for c in range(NCH):
                ci = pi * NCH + c
                st = st_cur
                out_ps = ps_out.tile([VW, 2 * CH], DT.float32, tag="out", name="out")
                qsl = slice(c * 4, c * 4 + 4)
                for t in range(NT):
                    # finalize previous chunk: flush its trailing PV first
                    # (the copy must observe all 16 accumulation steps!), then
                    # stage copies early so PE's PV(t0) isn't blocked long on
                    # the out accumulator.
                    if t == 0 and fin is not None:
                        while pending_pv:
                            emit_pv(*pending_pv.popleft())
                        fin2 = emit_fin_copy(fin)
                        if fin["c"] % 2 == 1:
                            fin2_done = fin2
                        else:
                            st["fin2_open"] = fin2
                        fin = None
                    scp = ps_sc.tile([P, 2 * CH], DT.float32, tag="sc", name="sc")
                    nc.tensor.matmul(
                        scp[:, :CH],
                        st["kT"][0:64, t, :],
                        st["qT"][0:64, qsl, :],
                        start=True, stop=True,
                        tile_position=(0, 0),
                    )
                    nc.tensor.matmul(
                        scp[:, CH:],
                        st["kT"][64:128, t, :],
                        st["qT"][64:128, qsl, :],
                        start=True, stop=True,
                        tile_position=(64, 0),
                    )
                    probs = pr.tile([P, 2 * CH], DT.bfloat16, tag="probs", name="probs")
                    if t in SCALAR_T:
                        nc.scalar.activation(probs[:], scp[:], AF.Exp)
                    else:
                        nc.vector.tensor_scalar(
                            probs[:].bitcast(DT.int16),
                            scp[:], A16, B16, OP.mult, OP.add,
                        )
                    if len(pending_pv) >= PVD:
                        emit_pv(*pending_pv.popleft())
                    pending_pv.append((st, out_ps, probs, t))

                    if t == 1 and fin2_done is not None:
                        emit_fin_tr(fin2_done)
                    if t == 3 and fin2_done is not None:
                        emit_fin_out(fin2_done)
                        fin2_done = None
                    # stream remaining setup: ~3 ops per even iteration
                    if pending_setup and t % 2 == 0:
                        for _ in range(3):
                            if pending_setup:
                                pending_setup.pop(0)()
                    # kick off next pair's setup in the 3rd chunk of this pair
                    if c == 2 and t == 0 and pi + 1 < len(PAIRS):
                        st_next = alloc_pair(pi + 1)
                        pending_setup = p1_stream_ops(st_next)

                # odd chunks inherit the open 2-chunk fin group
                fin = {"out_ps": out_ps, "pi": pi, "c": c}
                if c % 2 == 1:
                    fin["fin2"] = st.pop("fin2_open")
                if ci == len(PAIRS) * NCH - 1:
                    while pending_pv:
                        emit_pv(*pending_pv.popleft())
                    fin2 = emit_fin_copy(fin)
                    fin = None
                    emit_fin_tr(fin2)
                    emit_fin_out(fin2)

            st_cur = st_next
            st_next = None

    nc.compile()
    return nc


_NC = None


def _get_nc():
    global _NC
    if _NC is None:
        _NC = build()
    return _NC


def kernel(q: np.ndarray, k: np.ndarray, v: np.ndarray) -> np.ndarray:
    qf = np.ascontiguousarray(q, dtype=np.float32).reshape(B * H, S, D)
    kf = np.ascontiguousarray(k, dtype=np.float32).reshape(B * H, S, D)
    vf = np.ascontiguousarray(v, dtype=np.float32).reshape(B * H, S, D)
    in_maps = [
        {
            "q": qf[c * HPC : (c + 1) * HPC],
            "k": kf[c * HPC : (c + 1) * HPC],
            "v": vf[c * HPC : (c + 1) * HPC],
        }
        for c in range(NCORES)
    ]
    nc = _get_nc()
    res = run_bass_kernel_spmd(nc, in_maps, core_ids=list(range(NCORES)))
    out = np.concatenate([res.results[c]["out"] for c in range(NCORES)], axis=0)
    return out.reshape(B, H, S, D)


# revision 6
# speedup vs baseline: 1.4750x; 1.4750x over previous
"""Trainium2 Bass kernel (final: v4a + all-scalar exp) for nn_AttentionCore64: softmax(Q@K^T)@V (raw exp,
no scaling), B=2 H=16 S=2048 D=64, f32 in/out. B*H sharded over 8 cores.

v3 vs v2: DMA instruction count cut ~7x (~40 vs ~280; HWDGE descriptor-gen is
625ns/DMA on a single shared resource):
- XBAR transposes batched: one dma_start_transpose covers N 128-col blocks
  (out[:, j, :] = in[:, j*128:(j+1)*128].T blockwise semantics, verified).
- loads are per-head 3D DMAs covering many o-tile groups; the fp32->bf16
  convert does the head-interleave relayout (engine APs can be 4D).
- fin batched per 2 chunks: one XBAR transpose [80,2048]->[128,16,80], one
  normalize, one output DMA per head per 2 chunks.

v4: PV emission delayed PVD=3 iterations (deque). With delay 1 the pipeline
period was forced to >= sem+exp latency (~1.3us/t -> 166us total): PV(t) sat
on PE right behind QK(t+1) but had to wait for exp(t). With PVD=3 the exp
latency amortizes 3x; the 3 PVs pending at chunk end flush into the next
chunk's first 3 iterations (which have no PV of their own yet).
"""

import numpy as np
from contextlib import ExitStack

import concourse.tile as tile
import concourse.mybir as mybir
from concourse import bacc
from concourse.bass_utils import run_bass_kernel_spmd

B, H, S, D = 2, 16, 2048, 64
NCORES = 8
HPC = (B * H) // NCORES  # 4 heads per core

P = 128
CH = 512            # queries per chunk
NCH = S // CH       # 4 chunks per pair
NT = S // P         # 16 key tiles
NO = S // P         # 16 query o-tiles
DT = mybir.dt
AF = mybir.ActivationFunctionType
OP = mybir.AluOpType

PAIRS = [(0, 1), (2, 3)]
VW = 80             # padded PV weight cols: 64 v + 1 ones + 15 zeros

A16 = float(128.0 / np.log(2.0))
B16 = 16256.0 - 7.30  # calibrated: E[schraudolph/exp] = 1.0003

# exp engine assignment per t (16 per chunk): 14 ScalarE / 2 DVE.
# The 2 DVE tiles must be isolated (spacing >= ~5): consecutive DVE tiles
# serialize the PE's in-order PV chain behind the slower DVE exp latency.
SCALAR_T = frozenset(t for t in range(16) if t not in (5, 11))


def build(reps=None):
    nc = bacc.Bacc("TRN2", target_bir_lowering=False, debug=False)
    q_ext = nc.dram_tensor("q", [HPC, S, D], DT.float32, kind="ExternalInput").ap()
    k_ext = nc.dram_tensor("k", [HPC, S, D], DT.float32, kind="ExternalInput").ap()
    v_ext = nc.dram_tensor("v", [HPC, S, D], DT.float32, kind="ExternalInput").ap()
    out_ext = nc.dram_tensor("out", [HPC, S, D], DT.float32, kind="ExternalOutput").ap()

    with tile.TileContext(nc) as tc, ExitStack() as ctx:
        if reps is not None:
            ctx.enter_context(tc.For_i(0, reps))
        sb = ctx.enter_context(tc.tile_pool(name="sb", bufs=2))
        pr = ctx.enter_context(tc.tile_pool(name="pr", bufs=4))
        ps_sc = ctx.enter_context(tc.tile_pool(name="ps_sc", bufs=3, space="PSUM"))
        ps_out = ctx.enter_context(tc.tile_pool(name="ps_out", bufs=1, space="PSUM"))

        # ---------------- per-pair state ----------------
        def alloc_pair(pi):
            st = {"pi": pi}
            # nat tiles are HEAD-MAJOR so each load DMA is 3D (one head)
            for nm in ("q_nat", "k_nat", "v_nat"):
                st[nm] = sb.tile([P, 2, NO, D], DT.float32, tag=nm, name=nm)
            # bf tiles are O-MAJOR: [o, head, d] so a 128-col block = one
            # o-tile of the packed pair (relayout happens in the convert)
            for nm in ("qbf", "kbf"):
                st[nm] = sb.tile([P, NO, 2, D], DT.bfloat16, tag=nm, name=nm)
            for nm in ("qT", "kT"):
                st[nm] = sb.tile([P, NO, P], DT.bfloat16, tag=nm, name=nm)
            st["vx"] = sb.tile([P, NT, 2, VW], DT.bfloat16, tag="vx", name="vx")
            return st

        def load_head(st, which, hi, g0, gn):
            """One 3D DMA: o-tile groups [g0, g0+gn) of one head."""
            hA, _ = PAIRS[st["pi"]]
            ext = {"q": q_ext, "k": k_ext, "v": v_ext}[which]
            dst = st[{"q": "q_nat", "k": "k_nat", "v": "v_nat"}[which]]
            sl = slice(g0 * 4, (g0 + gn) * 4)
            nc.sync.dma_start(
                dst[:, hi, sl, :],
                ext[hA + hi].rearrange("(o p) d -> p o d", p=P)[:, sl, :],
            )

        def convert_groups(st, which, g0, gn, eng):
            """fp32 head-major -> bf16 o-major for groups [g0, g0+gn)."""
            src = st[{"q": "q_nat", "k": "k_nat"}[which]]
            dst = st[{"q": "qbf", "k": "kbf"}[which]]
            sl = slice(g0 * 4, (g0 + gn) * 4)
            s_ap = src[:, :, sl, :].rearrange("p h o d -> p o h d")
            if eng == "s":
                nc.scalar.copy(dst[:, sl, :, :], s_ap)
            else:
                nc.vector.tensor_copy(dst[:, sl, :, :], s_ap)

        def vx_groups(st, g0, gn):
            """Build vx = [v | 1 | zeros] bf16 for groups [g0, g0+gn) (GPSIMD)."""
            sl = slice(g0 * 4, (g0 + gn) * 4)
            for hi in (0, 1):
                nc.gpsimd.tensor_copy(
                    st["vx"][:, sl, hi, :D], st["v_nat"][:, hi, sl, :]
                )
            nc.gpsimd.memset(st["vx"][:, sl, :, D], 1.0)
            nc.gpsimd.memset(st["vx"][:, sl, :, D + 1 :], 0.0)

        def tr_groups(st, which, g0, gn):
            """One XBAR DMA transposing o-tiles [g0*4, (g0+gn)*4) blockwise."""
            src = st[{"q": "qbf", "k": "kbf"}[which]]
            dst = st[{"q": "qT", "k": "kT"}[which]]
            sl = slice(g0 * 4, (g0 + gn) * 4)
            nc.sync.dma_start_transpose(dst[:, sl, :], src[:, sl, :, :])

        # ---------------- finalization (per 2-chunk group) ----------------
        # outT col-blocks ordered [A-c0 | A-c1 | B-c0 | B-c1] so each head's
        # transposed o-tiles are contiguous for a single 3D output DMA.
        def emit_fin_copy(fin):
            out_ps = fin["out_ps"]
            cc = fin["c"] % 2
            if cc == 0:
                fin2 = {"pi": fin["pi"], "c0": fin["c"]}
                fin2["outT"] = sb.tile(
                    [VW, 2, 2, CH], DT.bfloat16, tag="outT", name="outT"
                )
            else:
                fin2 = fin["fin2"]
            outT = fin2["outT"]
            nc.scalar.copy(outT[:, 0, cc, :], out_ps[:, :CH])
            nc.vector.tensor_copy(outT[:, 1, cc, :], out_ps[:, CH:])
            return fin2

        def emit_fin_tr(fin2):
            finT = sb.tile([P, 2, 8, VW], DT.bfloat16, tag="finT", name="finT")
            nc.sync.dma_start_transpose(finT[:], fin2["outT"][:])
            fin2["finT"] = finT

        def emit_fin_out(fin2):
            finT = fin2["finT"]  # [128, hi, (cc j), 80]
            hA, _ = PAIRS[fin2["pi"]]
            c0 = fin2["c0"]
            recip = sb.tile([P, 2, 8, 1], DT.float32, tag="recip", name="recip")
            nc.vector.reciprocal(recip[:], finT[:, :, :, D : D + 1])
            outn = sb.tile([P, 2, 8, D], DT.float32, tag="outn", name="outn")
            nc.vector.tensor_tensor(
                outn[:],
                finT[:, :, :, :D],
                recip[:].to_broadcast((P, 2, 8, D)),
                OP.mult,
            )
            for hi in (0, 1):
                nc.sync.dma_start(
                    out_ext[hA + hi].rearrange("(o p) d -> p o d", p=P)[
                        :, c0 * 4 : c0 * 4 + 8, :
                    ],
                    outn[:, hi, :, :],
                )

        def emit_pv(st, out_ps, probs, t):
            nc.tensor.matmul(
                out_ps[:, :CH], st["vx"][:, t, 0, :], probs[:, :CH],
                start=(t == 0), stop=(t == NT - 1),
            )
            nc.tensor.matmul(
                out_ps[:, CH:], st["vx"][:, t, 1, :], probs[:, CH:],
                start=(t == 0), stop=(t == NT - 1),
            )

        # ---------------- pair-setup emission scheduling ----------------
        def p0_stream_ops(st):
            return [
                lambda: load_head(st, "k", 0, 2, 2),
                lambda: load_head(st, "k", 1, 2, 2),
                lambda: convert_groups(st, "k", 2, 1, "s"),
                lambda: convert_groups(st, "k", 3, 1, "v"),
                lambda: tr_groups(st, "k", 2, 2),
                lambda: load_head(st, "v", 0, 2, 2),
                lambda: load_head(st, "v", 1, 2, 2),
                lambda: vx_groups(st, 2, 1),
                lambda: vx_groups(st, 3, 1),
                lambda: load_head(st, "q", 0, 1, 3),
                lambda: load_head(st, "q", 1, 1, 3),
                lambda: convert_groups(st, "q", 1, 1, "v"),
                lambda: convert_groups(st, "q", 2, 1, "s"),
                lambda: convert_groups(st, "q", 3, 1, "v"),
                lambda: tr_groups(st, "q", 1, 3),
            ]

        def p1_stream_ops(st):
            return [
                lambda: load_head(st, "k", 0, 0, 4),
                lambda: load_head(st, "k", 1, 0, 4),
                lambda: convert_groups(st, "k", 0, 2, "s"),
                lambda: convert_groups(st, "k", 2, 2, "v"),
                lambda: tr_groups(st, "k", 0, 4),
                lambda: load_head(st, "v", 0, 0, 4),
                lambda: load_head(st, "v", 1, 0, 4),
                lambda: vx_groups(st, 0, 2),
                lambda: vx_groups(st, 2, 2),
                lambda: load_head(st, "q", 0, 0, 4),
                lambda: load_head(st, "q", 1, 0, 4),
                lambda: convert_groups(st, "q", 0, 2, "s"),
                lambda: convert_groups(st, "q", 2, 2, "v"),
                lambda: tr_groups(st, "q", 0, 4),
            ]

        # ---------------- prologue: pair 0 groups 0-1 of k/v, group 0 of q --
        st_cur = alloc_pair(0)
        load_head(st_cur, "k", 0, 0, 2)
        load_head(st_cur, "k", 1, 0, 2)
        load_head(st_cur, "q", 0, 0, 1)
        load_head(st_cur, "q", 1, 0, 1)
        convert_groups(st_cur, "k", 0, 1, "s")
        convert_groups(st_cur, "q", 0, 1, "v")
        tr_groups(st_cur, "q", 0, 1)
        convert_groups(st_cur, "k", 1, 1, "s")
        tr_groups(st_cur, "k", 0, 2)
        load_head(st_cur, "v", 0, 0, 2)
        load_head(st_cur, "v", 1, 0, 2)
        vx_groups(st_cur, 0, 2)
        pending_setup = p0_stream_ops(st_cur)

        from collections import deque

        PVD = 2
        st_next = None
        pending_pv = deque()
        fin = None
        fin2_done = None  # 2-chunk fin group ready for tr/out

        for pi in range(len(PAIRS)):
            for c in range(NCH):
                ci = pi * NCH + c
                st = st_cur
                out_ps = ps_out.tile([VW, 2 * CH], DT.float32, tag="out", name="out")
                qsl = slice(c * 4, c * 4 + 4)
                for t in range(NT):
                    # finalize previous chunk: flush its trailing PV first
                    # (the copy must observe all 16 accumulation steps!), then
                    # stage copies early so PE's PV(t0) isn't blocked long on
                    # the out accumulator.
                    if t == 0 and fin is not None:
                        while pending_pv:
                            emit_pv(*pending_pv.popleft()[1])
                        fin2 = emit_fin_copy(fin)
                        if fin["c"] % 2 == 1:
                            fin2_done = fin2
                        else:
                            st["fin2_open"] = fin2
                        fin = None
                    scp = ps_sc.tile([P, 2 * CH], DT.float32, tag="sc", name="sc")
                    nc.tensor.matmul(
                        scp[:, :CH],
                        st["kT"][0:64, t, :],
                        st["qT"][0:64, qsl, :],
                        start=True, stop=True,
                        tile_position=(0, 0),
                    )
                    nc.tensor.matmul(
                        scp[:, CH:],
                        st["kT"][64:128, t, :],
                        st["qT"][64:128, qsl, :],
                        start=True, stop=True,
                        tile_position=(64, 0),
                    )
                    probs = pr.tile([P, 2 * CH], DT.bfloat16, tag="probs", name="probs")
                    if t in SCALAR_T:
                        nc.scalar.activation(probs[:], scp[:], AF.Exp)
                    else:
                        nc.vector.tensor_scalar(
                            probs[:].bitcast(DT.int16),
                            scp[:], A16, B16, OP.mult, OP.add,
                        )
                    git = ci * NT + t
                    due = [e for e in pending_pv if e[0] <= git]
                    for e in due:
                        pending_pv.remove(e)
                        emit_pv(*e[1])
                    delay = 3 if t not in SCALAR_T else PVD
                    pending_pv.append((git + delay, (st, out_ps, probs, t)))

                    if t == 1 and fin2_done is not None:
                        emit_fin_tr(fin2_done)
                    if t == 3 and fin2_done is not None:
                        emit_fin_out(fin2_done)
                        fin2_done = None
                    # stream remaining setup: ~3 ops per even iteration
                    if pending_setup and t % 2 == 0:
                        for _ in range(3):
                            if pending_setup:
                                pending_setup.pop(0)()
                    # kick off next pair's setup in the 3rd chunk of this pair
                    if c == 2 and t == 0 and pi + 1 < len(PAIRS):
                        st_next = alloc_pair(pi + 1)
                        pending_setup = p1_stream_ops(st_next)

                # odd chunks inherit the open 2-chunk fin group
                fin = {"out_ps": out_ps, "pi": pi, "c": c}
                if c % 2 == 1:
                    fin["fin2"] = st.pop("fin2_open")
                if ci == len(PAIRS) * NCH - 1:
                    while pending_pv:
                        emit_pv(*pending_pv.popleft()[1])
                    fin2 = emit_fin_copy(fin)
                    fin = None
                    emit_fin_tr(fin2)
                    emit_fin_out(fin2)

            st_cur = st_next
            st_next = None

    nc.compile()
    return nc


_NC = None


def _get_nc():
    global _NC
    if _NC is None:
        _NC = build()
    return _NC


def kernel(q: np.ndarray, k: np.ndarray, v: np.ndarray) -> np.ndarray:
    qf = np.ascontiguousarray(q, dtype=np.float32).reshape(B * H, S, D)
    kf = np.ascontiguousarray(k, dtype=np.float32).reshape(B * H, S, D)
    vf = np.ascontiguousarray(v, dtype=np.float32).reshape(B * H, S, D)
    in_maps = [
        {
            "q": qf[c * HPC : (c + 1) * HPC],
            "k": kf[c * HPC : (c + 1) * HPC],
            "v": vf[c * HPC : (c + 1) * HPC],
        }
        for c in range(NCORES)
    ]
    nc = _get_nc()
    res = run_bass_kernel_spmd(nc, in_maps, core_ids=list(range(NCORES)))
    out = np.concatenate([res.results[c]["out"] for c in range(NCORES)], axis=0)
    return out.reshape(B, H, S, D)
